# revision 1
# baseline (speedup 1.0000x reference)
"""AttnBlock (GroupNorm -> 1x1 qkv -> softmax attention -> 1x1 proj -> residual)
for Trainium2, data-parallel over batch across 8 NeuronCores.

Shapes (hardcoded): x [8, 512, 2048] fp32. One batch element per core.
Measured ~50-70us/core steady-state (fp8 DoubleRow), rel err ~8e-4 vs fp32 ref.

Per-core algorithm (C=512, L=2048, P=128):
  - GroupNorm: 4 groups of 128 channels == one [128, 2048] SBUF tile each.
    Per-partition stats via bn_stats/bn_aggr; cross-partition reduce and
    broadcast via exact fp32 ones-matmuls on the PE (gpsimd custom ops do
    not compile in this toolchain). xn = x*A + B cast to fp8e4m3 into
    PAIRED tiles [128, 2(chunk), 2048]; x stays resident fp32 for the
    residual.
  - ALL big matmuls run fp8e4m3 with perf_mode=DoubleRow: contraction of
    256 per matmul via K-chunk pairs in the [K, 2, *] middle dim (weights
    host-pretransposed, host-paired, host-cast; activations paired at the
    PSUM->SBUF copyback, which casts for free). PSUM accumulates fp32.
  - Attention is computed TRANSPOSED: S^T[j,i] = K^T Q with j on
    partitions. exp (fused 1/sqrt(C) scale, max-subtraction skipped:
    logits are O(1)) reads 2-bank [128,1024] PSUM tiles and writes fp8 E.
  - Softmax normalizer: d = DoubleRow matmul with an all-ones [128,2,128]
    lhsT over E -> every PSUM row holds d (broadcast for free);
    dinv = reciprocal (DVE, fp32). The 1/d is folded into the O copyback
    (tensor_mul by dinv while casting O to fp8), keeping softmax weights
    summing to exactly 1 w.r.t. the rounded E used in O.
  - O_unnorm[c,i] = sum_j VT[j,c] E[j,i]; proj out2 = pw @ O_n; final
    fo = (out2 + pb_eff) + x in one fused scalar_tensor_tensor from PSUM.
    v-bias is folded into pb on the host: pb_eff = pb + pw@vb.
  - PSUM->SBUF copybacks are split between ScalarE and VectorE to balance
    engine load (ScalarE carries the irreducible exp work, ~38us).
"""

import numpy as np

import concourse.bass as bass
import concourse.mybir as mybir
import concourse.tile as tile
from concourse import bass_isa
from concourse.bass_utils import run_bass_kernel_spmd

F32 = mybir.dt.float32
BF16 = mybir.dt.bfloat16
FP8 = mybir.dt.float8e4
OSCALE = 1.0 / 64.0  # keeps O_unnorm within fp8/bf16 range; cancels via d

B = 8
C = 512
L = 2048
P = 128
GROUPS = 4
EPS = 1e-6
SCALE = float(C) ** -0.5

NCT = C // P  # 4 channel tiles
NLT = L // P  # 16 L tiles
IB = 512  # i-block width
NIB = L // IB  # 4 i blocks


def build_program(repeat=1):
    from concourse import bacc

    nc = bacc.Bacc("TRN2", target_bir_lowering=False, debug=False, num_devices=B)

    x_d = nc.dram_tensor("x", [C, L], F32, kind="ExternalInput").ap()
    w2_d = {
        p: nc.dram_tensor(f"{p}w2", [2, P, 2, C], FP8, kind="ExternalInput").ap()
        for p in ("q", "k", "v", "p")
    }
    qb_d = nc.dram_tensor("qb", [C], F32, kind="ExternalInput").ap()
    kb_d = nc.dram_tensor("kb", [C], F32, kind="ExternalInput").ap()
    pb_d = nc.dram_tensor("pb_eff", [C], F32, kind="ExternalInput").ap()
    gnw_d = nc.dram_tensor("gn_w", [C], F32, kind="ExternalInput").ap()
    gnb_d = nc.dram_tensor("gn_b", [C], F32, kind="ExternalInput").ap()
    out_d = nc.dram_tensor("out", [C, L], F32, kind="ExternalOutput").ap()

    from contextlib import ExitStack

    with tile.TileContext(nc) as tc, ExitStack() as ctx:
        pools = _make_pools(ctx, tc)
        for _ in range(repeat):
            _body(pools, tc, x_d, w2_d, qb_d, kb_d, pb_d, gnw_d, gnb_d, out_d)
    nc.compile()
    return nc


def _make_pools(ctx, tc):
    return {
        "consts": ctx.enter_context(tc.tile_pool(name="consts", bufs=1)),
        "persist": ctx.enter_context(tc.tile_pool(name="persist", bufs=1)),
        "xe": ctx.enter_context(tc.tile_pool(name="xe", bufs=12)),
        "small": ctx.enter_context(tc.tile_pool(name="small", bufs=4)),
        "osb": ctx.enter_context(tc.tile_pool(name="osb", bufs=10)),
        "fin": ctx.enter_context(tc.tile_pool(name="fin", bufs=4)),
        "dinv": ctx.enter_context(tc.tile_pool(name="dinv", bufs=3)),
        "ps": ctx.enter_context(tc.tile_pool(name="ps", bufs=3, space="PSUM")),
        "psd": ctx.enter_context(tc.tile_pool(name="psd", bufs=1, space="PSUM")),
        "psb": ctx.enter_context(tc.tile_pool(name="psb", bufs=1, space="PSUM")),
    }


def _body(pools, tc, x_d, w2_d, qb_d, kb_d, pb_d, gnw_d, gnb_d, out_d):
    nc = tc.nc
    Exp = mybir.ActivationFunctionType.Exp
    Identity = mybir.ActivationFunctionType.Identity
    Sqrt = mybir.ActivationFunctionType.Sqrt
    mult = mybir.AluOpType.mult
    add = mybir.AluOpType.add

    consts = pools["consts"]
    persist = pools["persist"]
    xe_pool = pools["xe"]
    small = pools["small"]
    osb_pool = pools["osb"]
    fin_pool = pools["fin"]
    dinv_pool = pools["dinv"]
    ps_pool = pools["ps"]
    psd_pool = pools["psd"]
    psb_pool = pools["psb"]

    # ---- constants ----
    w2 = {}
    for p in ("q", "k", "v", "p"):
        for pr in range(2):
            t = consts.tile([P, 2, C], FP8, name=f"w2_{p}_{pr}", tag=f"w2_{p}_{pr}", bufs=2)
            nc.sync.dma_start(out=t, in_=w2_d[p][pr])
            w2[(p, pr)] = t

    def load_cvec(name, src):
        t = consts.tile([P, NCT], F32, name=name, tag=name)
        for ct in range(NCT):
            nc.sync.dma_start(out=t[:, ct : ct + 1], in_=src[ct * P : (ct + 1) * P, None])
        return t

    qb_sb = load_cvec("qb_sb", qb_d)
    kb_sb = load_cvec("kb_sb", kb_d)
    pb_sb = load_cvec("pb_sb", pb_d)
    gnw_sb = load_cvec("gnw_sb", gnw_d)
    gnb_sb = load_cvec("gnb_sb", gnb_d)

    ones_bc = consts.tile([P, 2, P], FP8, name="ones_bc", tag="ones_bc")
    nc.vector.memset(ones_bc, 1.0)
    ones_col_f32 = consts.tile([P, 1], F32, name="ones_col_f32", tag="ones_col_f32")
    nc.vector.memset(ones_col_f32, 1.0)
    ones_row_f32 = consts.tile([1, P], F32, name="ones_row_f32", tag="ones_row_f32")
    nc.vector.memset(ones_row_f32, 1.0)
    eps_t = consts.tile([P, 1], F32, name="eps_t", tag="eps_t")
    nc.vector.memset(eps_t, EPS)

    # ---- load x (stays resident, fp32) + groupnorm into bf16 xn tiles ----
    x_sb = []
    for g in range(GROUPS):
        xg = persist.tile([P, L], F32, name=f"x_{g}", tag=f"x_{g}", bufs=2)
        nc.sync.dma_start(out=xg, in_=x_d[g * P : (g + 1) * P, :])
        x_sb.append(xg)

    xn = [
        xe_pool.tile([P, 2, L], FP8, tag="xe2", name=f"xn2_{p}", bufs=4)
        for p in range(2)
    ]
    for g in range(GROUPS):
        xg = x_sb[g]
        stats = small.tile([P, 4, 6], F32, name=f"gnstats_{g}", tag=f"gnstats_{g}", bufs=1)
        for s in range(4):
            nc.vector.bn_stats(out=stats[:, s, :], in_=xg[:, s * 512 : (s + 1) * 512])
        mv = small.tile([P, 2], F32, name=f"gnmv_{g}", tag=f"gnmv_{g}", bufs=1)
        nc.vector.bn_aggr(out=mv, in_=stats)
        # mv = [mean_p, var_p] per partition; mv[:,1] <- var_p + mean_p^2
        nc.vector.scalar_tensor_tensor(
            out=mv[:, 1:2], in0=mv[:, 0:1], scalar=mv[:, 0:1], in1=mv[:, 1:2],
            op0=mult, op1=add,
        )
        # cross-partition sum of [mean_p, m2_p] via exact fp32 ones-matmuls:
        # [128,2] -> [1,2] (reduce) -> [128,2] (broadcast)
        gsum_ps = psd_pool.tile([1, 2], F32, tag="d", name=f"gsum_ps_{g}")
        nc.tensor.matmul(gsum_ps, lhsT=ones_col_f32, rhs=mv, start=True, stop=True)
        gsum = small.tile([1, 2], F32, name=f"gsum_{g}", tag=f"gsum_{g}", bufs=1)
        nc.scalar.copy(gsum, gsum_ps)
        gbc_ps = psd_pool.tile([P, 2], F32, tag="d", name=f"gbc_ps_{g}")
        nc.tensor.matmul(gbc_ps, lhsT=ones_row_f32, rhs=gsum, start=True, stop=True)
        nc.scalar.copy(mv, gbc_ps)
        nc.vector.tensor_scalar_mul(mv, mv, 1.0 / P)  # [mean_g, E[x^2]_g]
        msq = small.tile([P, 1], F32, name=f"gnmsq_{g}", tag=f"gnmsq_{g}", bufs=1)
        nc.vector.tensor_mul(msq, mv[:, 0:1], mv[:, 0:1])
        varg = small.tile([P, 1], F32, name=f"gnvar_{g}", tag=f"gnvar_{g}", bufs=1)
        nc.vector.tensor_sub(varg, mv[:, 1:2], msq)
        stdg = small.tile([P, 1], F32, name=f"gnstd_{g}", tag=f"gnstd_{g}", bufs=1)
        nc.scalar.activation(stdg, varg, Sqrt, bias=eps_t)
        rstd = small.tile([P, 1], F32, name=f"gnrstd_{g}", tag=f"gnrstd_{g}", bufs=1)
        nc.vector.reciprocal(rstd, stdg)
        a_t = small.tile([P, 1], F32, name=f"gnA_{g}", tag=f"gnA_{g}", bufs=1)
        nc.vector.tensor_mul(a_t, rstd, gnw_sb[:, g : g + 1])
        ma_t = small.tile([P, 1], F32, name=f"gnmA_{g}", tag=f"gnmA_{g}", bufs=1)
        nc.vector.tensor_mul(ma_t, mv[:, 0:1], a_t)
        b_t = small.tile([P, 1], F32, name=f"gnB_{g}", tag=f"gnB_{g}", bufs=1)
        nc.vector.tensor_sub(b_t, gnb_sb[:, g : g + 1], ma_t)
        # xn = fp8(x*A + B), written into pair tile [128, 2, L]
        nc.vector.tensor_scalar(
            out=xn[g // 2][:, g % 2, :], in0=xg, scalar1=a_t, scalar2=b_t,
            op0=mult, op1=add,
        )

    # ---- Q, K as fp8 PAIRED tiles [c-pair][128, 2, L] for DoubleRow;
    #      VT as fp8 paired tiles [j-pair][128, 2, C] ----
    q2 = [persist.tile([P, 2, L], FP8, name=f"q2_{p}", tag=f"q2_{p}", bufs=2) for p in range(2)]
    k2 = [persist.tile([P, 2, L], FP8, name=f"k2_{p}", tag=f"k2_{p}", bufs=2) for p in range(2)]
    for ot in range(NCT):
        for pname, dest, bias in (("q", q2, qb_sb), ("k", k2, kb_sb)):
            t = dest[ot // 2]
            for lb in range(NIB):
                ps = ps_pool.tile([P, IB], F32, tag="ps", name=f"qk_ps_{pname}_{ot}_{lb}")
                for pr in range(2):
                    nc.tensor.matmul(
                        ps,
                        lhsT=w2[(pname, pr)][:, :, ot * P : (ot + 1) * P],
                        rhs=xn[pr][:, :, lb * IB : (lb + 1) * IB],
                        start=(pr == 0),
                        stop=(pr == 1),
                        perf_mode=mybir.MatmulPerfMode.DoubleRow,
                    )
                if (ot + lb) % 2 == 0:
                    nc.scalar.activation(
                        t[:, ot % 2, lb * IB : (lb + 1) * IB], ps, Identity,
                        bias=bias[:, ot : ot + 1],
                    )
                else:
                    nc.vector.tensor_scalar(
                        out=t[:, ot % 2, lb * IB : (lb + 1) * IB], in0=ps,
                        scalar1=bias[:, ot : ot + 1], scalar2=None, op0=add,
                    )

    vt2 = [
        persist.tile([P, 2, C], FP8, name=f"vt2_{p}", tag=f"vt2_{p}", bufs=2)
        for p in range(NLT // 2)
    ]
    for lt in range(NLT):
        ps = ps_pool.tile([P, C], F32, tag="ps", name=f"vt_ps_{lt}")
        for pr in range(2):
            nc.tensor.matmul(
                ps,
                lhsT=xn[pr][:, :, lt * P : (lt + 1) * P],
                rhs=w2[("v", pr)],
                start=(pr == 0),
                stop=(pr == 1),
                perf_mode=mybir.MatmulPerfMode.DoubleRow,
            )
        if lt % 2 == 0:
            nc.scalar.copy(vt2[lt // 2][:, lt % 2, :], ps)
        else:
            nc.vector.tensor_copy(vt2[lt // 2][:, lt % 2, :], ps)

    # ---- attention, i-block at a time ----
    for ib in range(NIB):
        isl = slice(ib * IB, (ib + 1) * IB)

        # E = exp(scale * K^T Q) fp8, transposed layout [j(part), i], packed
        # as 4 tiles [128, 2048] holding 4 j-tiles each. S^T via fp8 DoubleRow
        # (contraction c = 2 chunks of 256).
        e_pack = [
            xe_pool.tile([P, L], FP8, tag="xe", name=f"e_{ib}_{t}") for t in range(4)
        ]

        def e_view(jt):
            t, s = divmod(jt, 4)
            return e_pack[t][:, s * IB : (s + 1) * IB]

        def e_pair_view(jp):
            t, a = divmod(jp, 2)
            return e_pack[t][:, 2 * a * IB : 2 * (a + 1) * IB].rearrange(
                "p (s n) -> p s n", s=2
            )

        for t2 in range(NLT // 2):
            ps2b = ps_pool.tile([P, 2 * IB], F32, tag="ps2", bufs=2,
                                name=f"s_ps_{ib}_{t2}")
            for s in range(2):
                jt = 2 * t2 + s
                for p2 in range(2):
                    nc.tensor.matmul(
                        ps2b[:, s * IB : (s + 1) * IB],
                        lhsT=k2[p2][:, :, jt * P : (jt + 1) * P],
                        rhs=q2[p2][:, :, isl],
                        start=(p2 == 0),
                        stop=(p2 == 1),
                        perf_mode=mybir.MatmulPerfMode.DoubleRow,
                    )
            t, a = divmod(t2, 2)
            nc.scalar.activation(
                e_pack[t][:, 2 * a * IB : 2 * (a + 1) * IB], ps2b, Exp, scale=SCALE
            )

        # d[i] = sum_j E[j, i] * OSCALE via DoubleRow with an all-ones lhsT
        # [128, 2, 128] -> every psum row holds d (already broadcast), then
        # reciprocal straight from PSUM (per-free-elem cost is partition-
        # count independent on DVE).
        d_ps = psd_pool.tile([P, IB], F32, tag="d", name=f"d_ps_{ib}")
        for jp in range(NLT // 2):
            nc.tensor.matmul(
                d_ps,
                lhsT=ones_bc,
                rhs=e_pair_view(jp),
                start=(jp == 0),
                stop=(jp == NLT // 2 - 1),
                perf_mode=mybir.MatmulPerfMode.DoubleRow,
            )
        dinvb = dinv_pool.tile([P, IB], F32, tag="dinvb", name=f"dinvb_{ib}")
        nc.vector.reciprocal(dinvb, d_ps)

        # O_unnorm[c, i] = sum_j VT[j, c] E[j, i]  (fp8 DoubleRow over j pairs);
        # copyback scaled by 1/64 to keep bf16/psum ranges tame — cancels via
        # the 1/64 folded into ones_col (d is scaled identically).
        o2 = [
            osb_pool.tile([P, 2, IB], FP8, tag="osb", name=f"o2_{ib}_{p}")
            for p in range(2)
        ]
        for ct in range(NCT):
            ps = ps_pool.tile([P, IB], F32, tag="ps", name=f"o_ps_{ib}_{ct}")
            for jp in range(NLT // 2):
                nc.tensor.matmul(
                    ps,
                    lhsT=vt2[jp][:, :, ct * P : (ct + 1) * P],
                    rhs=e_pair_view(jp),
                    start=(jp == 0),
                    stop=(jp == NLT // 2 - 1),
                    perf_mode=mybir.MatmulPerfMode.DoubleRow,
                )
            nc.vector.tensor_mul(o2[ct // 2][:, ct % 2, :], ps, dinvb)

        # out2 = pw @ O_unnorm ; final = out2*dinv + pb_eff + x
        for ot in range(NCT):
            ps2 = ps_pool.tile([P, IB], F32, tag="ps", name=f"p_ps_{ib}_{ot}")
            for pr in range(2):
                nc.tensor.matmul(
                    ps2,
                    lhsT=w2[("p", pr)][:, :, ot * P : (ot + 1) * P],
                    rhs=o2[pr],
                    start=(pr == 0),
                    stop=(pr == 1),
                    perf_mode=mybir.MatmulPerfMode.DoubleRow,
                )
            fo = fin_pool.tile([P, IB], F32, tag="fo", name=f"fo_{ib}_{ot}")
            nc.vector.scalar_tensor_tensor(
                out=fo, in0=ps2, scalar=pb_sb[:, ot : ot + 1],
                in1=x_sb[ot][:, isl], op0=add, op1=add,
            )
            nc.sync.dma_start(out=out_d[ot * P : (ot + 1) * P, isl], in_=fo)


_NC_CACHE = None


def _get_program():
    global _NC_CACHE
    if _NC_CACHE is None:
        _NC_CACHE = build_program()
    return _NC_CACHE


def make_in_maps(x, gn_w, gn_b, qw, qb, kw, kb, vw, vb, pw, pb):
    import ml_dtypes

    f = np.float32
    f8 = ml_dtypes.float8_e4m3

    def pair_w(w):
        # w [Cout, Cin] -> wT [Cin, Cout] -> [2(pair), 128(k), 2(sub), Cout] fp8
        wT = np.asarray(w, f).T.reshape(2, 2, P, C).transpose(0, 2, 1, 3)
        return np.ascontiguousarray(wT.astype(f8))

    pb_eff = np.asarray(pb, f) + np.asarray(pw, f) @ np.asarray(vb, f)
    shared = {
        "qw2": pair_w(qw), "kw2": pair_w(kw), "vw2": pair_w(vw), "pw2": pair_w(pw),
        "qb": np.ascontiguousarray(np.asarray(qb, f)),
        "kb": np.ascontiguousarray(np.asarray(kb, f)),
        "pb_eff": np.ascontiguousarray(pb_eff),
        "gn_w": np.ascontiguousarray(np.asarray(gn_w, f)),
        "gn_b": np.ascontiguousarray(np.asarray(gn_b, f)),
    }
    x = np.asarray(x, f)
    return [{"x": np.ascontiguousarray(x[b]), **shared} for b in range(B)]


def kernel(x, gn_w, gn_b, qw, qb, kw, kb, vw, vb, pw, pb):
    nc = _get_program()
    in_maps = make_in_maps(x, gn_w, gn_b, qw, qb, kw, kb, vw, vb, pw, pb)
    res = run_bass_kernel_spmd(nc, in_maps, core_ids=list(range(B)))
    return np.stack([res.results[b]["out"] for b in range(B)]).astype(np.float32)



# revision 3
# speedup vs baseline: 2.9689x; 2.9689x over previous
"""AttnBlock (GroupNorm -> 1x1 qkv -> softmax attention -> 1x1 proj -> residual)
for Trainium2, data-parallel over batch across 8 NeuronCores. v2.

Shapes (hardcoded): x [8, 512, 2048] fp32. One batch element per core.

Structure (per core; C=512, L=2048, P=128; all big matmuls fp8 DoubleRow,
N=512 free, PSUM fp32):
  - One packed DMA for all channel vectors (qb/pb_eff/gn_w/gn_b pre-laid-out
    [128,4,4] on host) and one 1MB DMA for all 8 pre-paired fp8 weight tiles;
    x arrives as 4 [128,2048] DMAs.
  - GroupNorm: bn_stats/bn_aggr per group; ONE fp32 ones-matmul pair reduces
    + broadcasts all 4 groups' [mean, m2] at once; [128,4]-wide elementwise
    chain produces per-group A,B; xn = fp8(x*A+B), x stays resident fp32.
  - k-bias dropped (softmax-invariant); v-bias folded into pb on host.
  - S^T = K^T Q computed j-tile-major; exp on ScalarE -> fp8 E pair tiles
    e2[jp] [128,2,2048]. V^T matmuls are interleaved into the S loop: PE
    stream per jt = 8 S + 2 V matmuls (2.4us) > exp drain (2.1us), so
    ScalarE (which does ONLY exp during S) never stalls the PE.
  - d = sum_j E via 32 DoubleRow matmuls with a single stationary ones tile
    (no weight reloads); every psum row holds d broadcast; reciprocal on DVE.
  - O_unnorm = VT E, i-half-major per ct so psum tiles free early; copyback
    folds 1/d and casts fp8. proj + bias + residual fused in one DVE op.
  - PSUM: tags [128,1024]x3 + [128,512]x2 = all 8 banks.
  - Repeat builds are software-pipelined: iteration r+1's x-DMA + bn_stats
    are emitted after QK(r) (run during S(r) when DVE is light), and its
    reduce/A,B/casts after S(r), so the PE never idles at the iteration
    boundary waiting for GroupNorm.
  - First iteration only: dummy DoubleRow matmuls during the x DMA lift the
    PE HAM clock gate to 8/8 before real matmuls start.
"""

import numpy as np

import concourse.bass as bass
import concourse.mybir as mybir
import concourse.tile as tile
from concourse.bass_utils import run_bass_kernel_spmd

F32 = mybir.dt.float32
BF16 = mybir.dt.bfloat16
FP8 = mybir.dt.float8e4
DR = mybir.MatmulPerfMode.DoubleRow

B = 8
C = 512
L = 2048
P = 128
GROUPS = 4
EPS = 1e-6
SCALE = float(C) ** -0.5

NCT = C // P  # 4 channel tiles
NLT = L // P  # 16 L tiles
N_WARM = 32
PROJ_IDX = {"q": 0, "k": 1, "v": 2, "p": 3}


def build_program(repeat=1):
    from concourse import bacc

    nc = bacc.Bacc("TRN2", target_bir_lowering=False, debug=False, num_devices=B)

    x_d = nc.dram_tensor("x", [C, L], F32, kind="ExternalInput").ap()
    wall_d = nc.dram_tensor("wall", [P, 4, 2, 2, C], FP8, kind="ExternalInput").ap()
    cv_d = nc.dram_tensor("cvecs", [P, 4, NCT], F32, kind="ExternalInput").ap()
    out_d = nc.dram_tensor("out", [C, L], F32, kind="ExternalOutput").ap()
    scratch_d = nc.dram_tensor("scratch", [P, C], F32, kind="ExternalOutput").ap()

    from contextlib import ExitStack

    with tile.TileContext(nc) as tc, ExitStack() as ctx:
        k = _Kern(_make_pools(ctx, tc), tc, x_d, wall_d, cv_d, out_d, scratch_d)
        st = k.head_dma_stats(0, warm=True)
        k.head_post(st)
        for r in range(repeat):
            k.attn_qk(st)
            st_next = k.head_dma_stats(r + 1, warm=False) if r + 1 < repeat else None
            k.attn_s(st)
            if st_next is not None:
                k.head_post(st_next)
            k.attn_rest(st)
            st = st_next
    nc.compile()
    return nc


def _make_pools(ctx, tc):
    return {
        "consts": ctx.enter_context(tc.tile_pool(name="consts", bufs=1)),
        "persist": ctx.enter_context(tc.tile_pool(name="persist", bufs=1)),
        "small": ctx.enter_context(tc.tile_pool(name="small", bufs=2)),
        "fin": ctx.enter_context(tc.tile_pool(name="fin", bufs=6)),
        "ps": ctx.enter_context(tc.tile_pool(name="ps", bufs=1, space="PSUM")),
    }


class _Kern:
    def __init__(self, pools, tc, x_d, wall_d, cv_d, out_d, scratch_d):
        self.pools = pools
        self.tc = tc
        self.nc = tc.nc
        self.x_d = x_d
        self.wall_d = wall_d
        self.cv_d = cv_d
        self.out_d = out_d
        self.scratch_d = scratch_d
        self._consts = None

    def psum2(self, name):
        return self.pools["ps"].tile([P, 1024], F32, tag="ps2", name=name, bufs=3)

    def psum1(self, name):
        return self.pools["ps"].tile([P, 512], F32, tag="ps1", name=name, bufs=2)

    # ---- constants (per iteration: one weight DMA, one cvec DMA) ----
    def consts(self, r):
        nc = self.nc
        consts = self.pools["consts"]
        wall = consts.tile([P, 4, 2, 2, C], FP8, name=f"wall_{r}", tag="wall", bufs=2)
        nc.sync.dma_start(out=wall, in_=self.wall_d)
        cv = consts.tile([P, 4, NCT], F32, name=f"cv_{r}", tag="cv", bufs=2)
        nc.sync.dma_start(out=cv, in_=self.cv_d)
        if self._consts is None:
            ones_bc = consts.tile([P, 2, P], FP8, name="ones_bc", tag="ones_bc")
            nc.vector.memset(ones_bc, 1.0)
            ones_col = consts.tile([P, 1], F32, name="ones_col", tag="ones_col")
            nc.vector.memset(ones_col, 1.0)
            ones_row = consts.tile([1, P], F32, name="ones_row", tag="ones_row")
            nc.vector.memset(ones_row, 1.0)
            eps_t = consts.tile([P, 1], F32, name="eps_t", tag="eps_t")
            nc.vector.memset(eps_t, EPS)
            self._consts = (ones_bc, ones_col, ones_row, eps_t)
        return {"wall": wall, "cv": cv}

    # ---- head part 1: x DMA + per-partition stats (DVE work that runs
    # during the previous iteration's S phase) ----
    def head_dma_stats(self, r, warm):
        nc = self.nc
        persist = self.pools["persist"]
        small = self.pools["small"]
        st = self.consts(r)

        if warm:
            # PE warm-up: dummy DoubleRow matmuls during the x DMA keep the
            # HAM activity window busy so real matmuls start at 2.4 GHz.
            consts = self.pools["consts"]
            wtile = consts.tile([P, 2, C], FP8, name="wtile", tag="wtile")
            nc.vector.memset(wtile, 0.03125)
            wps = None
            for i in range(N_WARM):
                wps = self.psum1(f"warm_ps_{i}")
                nc.tensor.matmul(
                    wps, lhsT=wtile[:, :, 0:P], rhs=wtile,
                    start=True, stop=True, perf_mode=DR,
                )
            wsb = small.tile([P, C], F32, name="warm_sb", tag="warm_sb", bufs=1)
            nc.scalar.copy(wsb, wps)
            nc.sync.dma_start(out=self.scratch_d, in_=wsb)

        x_sb = []
        for g in range(GROUPS):
            xg = persist.tile([P, L], F32, name=f"x_{g}_{r}", tag=f"x_{g}", bufs=2)
            nc.sync.dma_start(out=xg, in_=self.x_d[g * P : (g + 1) * P, :])
            x_sb.append(xg)
        st["x_sb"] = x_sb

        mult = mybir.AluOpType.mult
        add = mybir.AluOpType.add
        mv_all = small.tile([P, GROUPS, 2], F32, name=f"gnmv_all_{r}", tag="gnmv_all")
        for g in range(GROUPS):
            stats = small.tile([P, 4, 6], F32, name=f"gnstats_{g}_{r}", tag=f"gnstats_{g}")
            for s in range(4):
                nc.vector.bn_stats(out=stats[:, s, :], in_=x_sb[g][:, s * 512 : (s + 1) * 512])
            mv = mv_all[:, g, :]
            nc.vector.bn_aggr(out=mv, in_=stats)
            # mv = [mean_p, var_p] per partition; mv[:,1] <- var_p + mean_p^2
            nc.vector.scalar_tensor_tensor(
                out=mv[:, 1:2], in0=mv[:, 0:1], scalar=mv[:, 0:1], in1=mv[:, 1:2],
                op0=mult, op1=add,
            )
        st["mv_all"] = mv_all
        return st

    # ---- head part 2: cross-partition reduce (one matmul pair), A/B chain,
    # fp8 casts ----
    def head_post(self, st):
        nc = self.nc
        small = self.pools["small"]
        persist = self.pools["persist"]
        ones_bc, ones_col, ones_row, eps_t = self._consts
        Sqrt = mybir.ActivationFunctionType.Sqrt
        mult = mybir.AluOpType.mult
        add = mybir.AluOpType.add
        cv = st["cv"]
        gnw_sb = cv[:, 2, :]
        gnb_sb = cv[:, 3, :]

        gsum_ps = self.psum1("gsum_ps")
        nc.tensor.matmul(gsum_ps[0:1, 0:8], lhsT=ones_col,
                         rhs=st["mv_all"].rearrange("p g two -> p (g two)"),
                         start=True, stop=True)
        gsum = small.tile([1, 8], F32, name="gsum", tag="gsum")
        nc.scalar.copy(gsum, gsum_ps[0:1, 0:8])
        gbc_ps = self.psum1("gbc_ps")
        nc.tensor.matmul(gbc_ps[:, 0:8], lhsT=ones_row, rhs=gsum, start=True, stop=True)
        mvb = small.tile([P, 8], F32, name="gnmvb", tag="gnmvb")
        nc.vector.tensor_scalar_mul(mvb, gbc_ps[:, 0:8], 1.0 / P)
        mean_a = mvb[:, 0:8:2]  # [128, 4]
        m2_a = mvb[:, 1:8:2]
        msq = small.tile([P, 4], F32, name="gnmsq", tag="gnmsq")
        nc.vector.tensor_mul(msq, mean_a, mean_a)
        varg = small.tile([P, 4], F32, name="gnvar", tag="gnvar")
        nc.vector.tensor_sub(varg, m2_a, msq)
        stdg = small.tile([P, 4], F32, name="gnstd", tag="gnstd")
        nc.scalar.activation(stdg, varg, Sqrt, bias=eps_t)
        rstd = small.tile([P, 4], F32, name="gnrstd", tag="gnrstd")
        nc.vector.reciprocal(rstd, stdg)
        a_t = small.tile([P, 4], F32, name="gnA", tag="gnA")
        nc.vector.tensor_mul(a_t, rstd, gnw_sb)
        ma_t = small.tile([P, 4], F32, name="gnmA", tag="gnmA")
        nc.vector.tensor_mul(ma_t, mean_a, a_t)
        b_t = small.tile([P, 4], F32, name="gnB", tag="gnB")
        nc.vector.tensor_sub(b_t, gnb_sb, ma_t)
        xn = [
            persist.tile([P, 2, L], FP8, tag=f"xn_{p}", name=f"xn2_{p}", bufs=2)
            for p in range(2)
        ]
        for g in range(GROUPS):
            # xn = fp8(x*A + B), written into pair tile [128, 2, L]
            nc.vector.tensor_scalar(
                out=xn[g // 2][:, g % 2, :], in0=st["x_sb"][g],
                scalar1=a_t[:, g : g + 1], scalar2=b_t[:, g : g + 1],
                op0=mult, op1=add,
            )
        st["xn"] = xn

    # ---- Q, K: w-stationary; ScalarE takes the plain k copies (idle pre-S),
    # DVE the q bias-add, keeping ScalarE exp-only during the S phase ----
    def attn_qk(self, st):
        nc = self.nc
        persist = self.pools["persist"]
        add = mybir.AluOpType.add
        wall, cv, xn = st["wall"], st["cv"], st["xn"]
        qb_sb = cv[:, 0, :]

        def w2(pname, pr):
            return wall[:, PROJ_IDX[pname], pr]

        st["w2"] = w2
        q2 = [persist.tile([P, 2, L], FP8, name=f"q2_{p}", tag=f"q2_{p}", bufs=1) for p in range(2)]
        k2 = [persist.tile([P, 2, L], FP8, name=f"k2_{p}", tag=f"k2_{p}", bufs=1) for p in range(2)]
        for ot in range(NCT):
            for pname, dest in (("q", q2), ("k", k2)):
                t = dest[ot // 2]
                for half in range(2):
                    ps = self.psum2(f"qk_ps_{pname}_{ot}_{half}")
                    for pr in range(2):
                        for s in range(2):
                            nc.tensor.matmul(
                                ps[:, s * 512 : (s + 1) * 512],
                                lhsT=w2(pname, pr)[:, :, ot * P : (ot + 1) * P],
                                rhs=xn[pr][:, :, (2 * half + s) * 512 : (2 * half + s + 1) * 512],
                                start=(pr == 0),
                                stop=(pr == 1),
                                perf_mode=DR,
                            )
                    dst = t[:, ot % 2, half * 1024 : (half + 1) * 1024]
                    if pname == "q":
                        nc.vector.tensor_scalar(
                            out=dst, in0=ps, scalar1=qb_sb[:, ot : ot + 1],
                            scalar2=None, op0=add,
                        )
                    else:
                        nc.scalar.copy(dst, ps)
        st["q2"], st["k2"] = q2, k2

    # ---- S^T = K^T Q per j-tile + exp; V^T matmuls interleaved as PE filler ----
    def attn_s(self, st):
        nc = self.nc
        persist = self.pools["persist"]
        Exp = mybir.ActivationFunctionType.Exp
        q2, k2, xn, w2 = st["q2"], st["k2"], st["xn"], st["w2"]

        vt2 = [
            persist.tile([P, 2, C], FP8, name=f"vt2_{p}", tag=f"vt2_{p}", bufs=1)
            for p in range(NLT // 2)
        ]
        e2 = [
            persist.tile([P, 2, L], FP8, tag=f"e2_{jp}", name=f"e2_{jp}", bufs=1)
            for jp in range(NLT // 2)
        ]
        for jt in range(NLT):
            jp, sub = divmod(jt, 2)
            for half in range(2):
                ps = self.psum2(f"s_ps_{jt}_{half}")
                for pr in range(2):
                    for s in range(2):
                        nc.tensor.matmul(
                            ps[:, s * 512 : (s + 1) * 512],
                            lhsT=k2[pr][:, :, jt * P : (jt + 1) * P],
                            rhs=q2[pr][:, :, (2 * half + s) * 512 : (2 * half + s + 1) * 512],
                            start=(pr == 0),
                            stop=(pr == 1),
                            perf_mode=DR,
                        )
                nc.scalar.activation(
                    e2[jp][:, sub, half * 1024 : (half + 1) * 1024], ps, Exp, scale=SCALE
                )
            # V^T for l-tile jt: 2 matmuls into a [128,512] psum, then copyback
            psv = self.psum1(f"vt_ps_{jt}")
            for pr in range(2):
                nc.tensor.matmul(
                    psv,
                    lhsT=xn[pr][:, :, jt * P : (jt + 1) * P],
                    rhs=w2("v", pr),
                    start=(pr == 0),
                    stop=(pr == 1),
                    perf_mode=DR,
                )
            nc.vector.tensor_copy(vt2[jp][:, sub, :], psv)
        st["vt2"], st["e2"] = vt2, e2

    # ---- d, O, proj + bias + residual + store ----
    def attn_rest(self, st):
        nc = self.nc
        persist = self.pools["persist"]
        fin_pool = self.pools["fin"]
        ones_bc = self._consts[0]
        mult = mybir.AluOpType.mult
        add = mybir.AluOpType.add
        e2, vt2, w2, cv = st["e2"], st["vt2"], st["w2"], st["cv"]
        pb_sb = cv[:, 1, :]

        # d[i] = sum_j E[j,i]: DoubleRow ones-matmuls, ones tile stationary
        # across all 32; every psum row holds d broadcast; reciprocal on DVE.
        dinvb = persist.tile([P, L], F32, tag="dinvb", name="dinvb", bufs=1)
        for ib in range(4):
            dps = self.psum1(f"d_ps_{ib}")
            for jp in range(NLT // 2):
                nc.tensor.matmul(
                    dps,
                    lhsT=ones_bc,
                    rhs=e2[jp][:, :, ib * 512 : (ib + 1) * 512],
                    start=(jp == 0),
                    stop=(jp == NLT // 2 - 1),
                    perf_mode=DR,
                )
            nc.vector.reciprocal(dinvb[:, ib * 512 : (ib + 1) * 512], dps)

        # O_unnorm[c, i] = sum_j VT[j, c] E[j, i], i-half-major per ct;
        # copyback folds 1/d and casts to fp8
        o2 = [persist.tile([P, 2, L], FP8, tag=f"o2_{p}", name=f"o2_{p}", bufs=1) for p in range(2)]
        for ct in range(NCT):
            t = o2[ct // 2]
            for half in range(2):
                po = self.psum2(f"o_ps_{ct}_{half}")
                for jp in range(NLT // 2):
                    for s in range(2):
                        nc.tensor.matmul(
                            po[:, s * 512 : (s + 1) * 512],
                            lhsT=vt2[jp][:, :, ct * P : (ct + 1) * P],
                            rhs=e2[jp][:, :, (2 * half + s) * 512 : (2 * half + s + 1) * 512],
                            start=(jp == 0),
                            stop=(jp == NLT // 2 - 1),
                            perf_mode=DR,
                        )
                nc.vector.tensor_mul(
                    t[:, ct % 2, half * 1024 : (half + 1) * 1024],
                    po,
                    dinvb[:, half * 1024 : (half + 1) * 1024],
                )

        # proj + bias + residual + store
        for ot in range(NCT):
            for half in range(2):
                pp = self.psum2(f"p_ps_{ot}_{half}")
                for pr in range(2):
                    for s in range(2):
                        nc.tensor.matmul(
                            pp[:, s * 512 : (s + 1) * 512],
                            lhsT=w2("p", pr)[:, :, ot * P : (ot + 1) * P],
                            rhs=o2[pr][:, :, (2 * half + s) * 512 : (2 * half + s + 1) * 512],
                            start=(pr == 0),
                            stop=(pr == 1),
                            perf_mode=DR,
                        )
                fo = fin_pool.tile([P, 1024], F32, tag="fo", name=f"fo_{ot}_{half}")
                nc.vector.scalar_tensor_tensor(
                    out=fo, in0=pp, scalar=pb_sb[:, ot : ot + 1],
                    in1=st["x_sb"][ot][:, half * 1024 : (half + 1) * 1024],
                    op0=add, op1=add,
                )
                nc.sync.dma_start(
                    out=self.out_d[ot * P : (ot + 1) * P, half * 1024 : (half + 1) * 1024],
                    in_=fo,
                )


_NC_CACHE = None


def _get_program():
    global _NC_CACHE
    if _NC_CACHE is None:
        _NC_CACHE = build_program()
    return _NC_CACHE


def make_in_maps(x, gn_w, gn_b, qw, qb, kw, kb, vw, vb, pw, pb):
    import ml_dtypes

    f = np.float32
    f8 = ml_dtypes.float8_e4m3

    def pair_w(w):
        # w [Cout, Cin] -> wT [Cin, Cout] -> [2(pair), 128(k), 2(sub), Cout] fp8
        wT = np.asarray(w, f).T.reshape(2, 2, P, C).transpose(0, 2, 1, 3)
        return wT.astype(f8)

    # all weights in one [128, 4(proj), 2(pair), 2(sub), C] tensor
    wall = np.stack(
        [pair_w(qw), pair_w(kw), pair_w(vw), pair_w(pw)], axis=0
    ).transpose(2, 0, 1, 3, 4)  # [P, proj, pr, sub, C]

    pb_eff = np.asarray(pb, f) + np.asarray(pw, f) @ np.asarray(vb, f)
    # channel vectors packed [128, 4(vec), 4(group-of-128)]
    cvecs = np.stack(
        [np.asarray(qb, f), pb_eff, np.asarray(gn_w, f), np.asarray(gn_b, f)], axis=0
    ).reshape(4, NCT, P).transpose(2, 0, 1)

    shared = {
        "wall": np.ascontiguousarray(wall),
        "cvecs": np.ascontiguousarray(cvecs),
    }
    x = np.asarray(x, f)
    return [{"x": np.ascontiguousarray(x[b]), **shared} for b in range(B)]


def kernel(x, gn_w, gn_b, qw, qb, kw, kb, vw, vb, pw, pb):
    nc = _get_program()
    in_maps = make_in_maps(x, gn_w, gn_b, qw, qb, kw, kb, vw, vb, pw, pb)
    res = run_bass_kernel_spmd(nc, in_maps, core_ids=list(range(B)))
    return np.stack([res.results[b]["out"] for b in range(B)]).astype(np.float32)


# revision 4
# speedup vs baseline: 11.2467x; 3.7882x over previous
"""AttnBlock (GroupNorm -> 1x1 qkv -> softmax attention -> 1x1 proj -> residual)
for Trainium2, data-parallel over batch across 8 NeuronCores. v2.

Shapes (hardcoded): x [8, 512, 2048] fp32. One batch element per core.

Structure (per core; C=512, L=2048, P=128; all big matmuls fp8 DoubleRow,
N=512 free, PSUM fp32):
  - One packed DMA for all channel vectors (qb/pb_eff/gn_w/gn_b pre-laid-out
    [128,4,4] on host) and one 1MB DMA for all 8 pre-paired fp8 weight tiles;
    x arrives as 4 [128,2048] DMAs.
  - GroupNorm: bn_stats/bn_aggr per group; ONE fp32 ones-matmul pair reduces
    + broadcasts all 4 groups' [mean, m2] at once; [128,4]-wide elementwise
    chain produces per-group A,B; xn = fp8(x*A+B), x stays resident fp32.
  - k-bias dropped (softmax-invariant); v-bias folded into pb on host.
  - S^T = K^T Q computed j-tile-major; exp on ScalarE -> fp8 E pair tiles
    e2[jp] [128,2,2048]. V^T matmuls are interleaved into the S loop: PE
    stream per jt = 8 S + 2 V matmuls (2.4us) > exp drain (2.1us), so
    ScalarE (which does ONLY exp during S) never stalls the PE.
  - d = sum_j E via 32 DoubleRow matmuls with a single stationary ones tile
    (no weight reloads); every psum row holds d broadcast; reciprocal on DVE.
  - O_unnorm = VT E, i-half-major per ct so psum tiles free early; copyback
    folds 1/d and casts fp8. proj + bias + residual fused in one DVE op.
  - PSUM: tags [128,1024]x3 + [128,512]x2 = all 8 banks.
  - Repeat builds are software-pipelined: iteration r+1's x-DMA + bn_stats
    are emitted after QK(r) (run during S(r) when DVE is light), and its
    reduce/A,B/casts after S(r), so the PE never idles at the iteration
    boundary waiting for GroupNorm.
  - First iteration only: dummy DoubleRow matmuls during the x DMA lift the
    PE HAM clock gate to 8/8 before real matmuls start.
"""

import numpy as np

import concourse.bass as bass
import concourse.mybir as mybir
import concourse.tile as tile
from concourse.bass_utils import run_bass_kernel_spmd

F32 = mybir.dt.float32
BF16 = mybir.dt.bfloat16
FP8 = mybir.dt.float8e4
DR = mybir.MatmulPerfMode.DoubleRow

B = 8
C = 512
L = 2048
P = 128
GROUPS = 4
EPS = 1e-6
SCALE = float(C) ** -0.5

NCT = C // P  # 4 channel tiles
NLT = L // P  # 16 L tiles
N_WARM = 56  # 56 x 243ns ~ 13.6us: covers the x-DMA + GroupNorm head so the
# HAM clock gate stays at 8/8 from the first real matmul
PROJ_IDX = {"q": 0, "k": 1, "v": 2, "p": 3}


def build_program(repeat=1):
    from concourse import bacc

    nc = bacc.Bacc("TRN2", target_bir_lowering=False, debug=False, num_devices=B)

    x_d = nc.dram_tensor("x", [C, L], F32, kind="ExternalInput").ap()
    wall_d = nc.dram_tensor("wall", [P, 4, 2, 2, C], FP8, kind="ExternalInput").ap()
    cv_d = nc.dram_tensor("cvecs", [P, 4, NCT], F32, kind="ExternalInput").ap()
    out_d = nc.dram_tensor("out", [C, L], F32, kind="ExternalOutput").ap()
    scratch_d = nc.dram_tensor("scratch", [P, C], F32, kind="ExternalOutput").ap()

    from contextlib import ExitStack

    with tile.TileContext(nc) as tc, ExitStack() as ctx:
        k = _Kern(_make_pools(ctx, tc), tc, x_d, wall_d, cv_d, out_d, scratch_d)
        st = k.head_dma_stats(0, warm=True)
        k.head_post(st)
        for r in range(repeat):
            k.attn_qk(st)
            st_next = k.head_dma_stats(r + 1, warm=False) if r + 1 < repeat else None
            k.attn_s(st)
            if st_next is not None:
                k.head_post(st_next)
            k.attn_rest(st)
            st = st_next
    nc.compile()
    return nc


def _make_pools(ctx, tc):
    return {
        "consts": ctx.enter_context(tc.tile_pool(name="consts", bufs=1)),
        "persist": ctx.enter_context(tc.tile_pool(name="persist", bufs=1)),
        "small": ctx.enter_context(tc.tile_pool(name="small", bufs=2)),
        "fin": ctx.enter_context(tc.tile_pool(name="fin", bufs=6)),
        "ps": ctx.enter_context(tc.tile_pool(name="ps", bufs=1, space="PSUM")),
    }


class _Kern:
    def __init__(self, pools, tc, x_d, wall_d, cv_d, out_d, scratch_d):
        self.pools = pools
        self.tc = tc
        self.nc = tc.nc
        self.x_d = x_d
        self.wall_d = wall_d
        self.cv_d = cv_d
        self.out_d = out_d
        self.scratch_d = scratch_d
        self._consts = None

    def psum2(self, name):
        return self.pools["ps"].tile([P, 1024], F32, tag="ps2", name=name, bufs=3)

    def psum1(self, name):
        return self.pools["ps"].tile([P, 512], F32, tag="ps1", name=name, bufs=2)

    # ---- constants (per iteration: one weight DMA, one cvec DMA) ----
    def consts(self, r):
        nc = self.nc
        consts = self.pools["consts"]
        wall = consts.tile([P, 4, 2, 2, C], FP8, name=f"wall_{r}", tag="wall", bufs=2)
        nc.sync.dma_start(out=wall, in_=self.wall_d)
        cv = consts.tile([P, 4, NCT], F32, name=f"cv_{r}", tag="cv", bufs=2)
        nc.sync.dma_start(out=cv, in_=self.cv_d)
        if self._consts is None:
            ones_bc = consts.tile([P, 2, P], FP8, name="ones_bc", tag="ones_bc")
            nc.vector.memset(ones_bc, 1.0)
            ones_col = consts.tile([P, 1], F32, name="ones_col", tag="ones_col")
            nc.vector.memset(ones_col, 1.0)
            ones_row = consts.tile([1, P], F32, name="ones_row", tag="ones_row")
            nc.vector.memset(ones_row, 1.0)
            eps_t = consts.tile([P, 1], F32, name="eps_t", tag="eps_t")
            nc.vector.memset(eps_t, EPS)
            self._consts = (ones_bc, ones_col, ones_row, eps_t)
        return {"wall": wall, "cv": cv}

    # ---- head part 1: x DMA + per-partition stats (DVE work that runs
    # during the previous iteration's S phase) ----
    def head_dma_stats(self, r, warm):
        nc = self.nc
        persist = self.pools["persist"]
        small = self.pools["small"]
        st = self.consts(r)

        x_sb = []
        for g in range(GROUPS):
            xg = persist.tile([P, L], F32, name=f"x_{g}_{r}", tag=f"x_{g}", bufs=2)
            nc.sync.dma_start(out=xg, in_=self.x_d[g * P : (g + 1) * P, :])
            x_sb.append(xg)
        st["x_sb"] = x_sb

        if warm:
            # PE warm-up: dummy DoubleRow matmuls during the x DMA keep the
            # HAM activity window busy so real matmuls start at 2.4 GHz.
            # Emitted after the x dma_starts so the transfers begin first.
            consts = self.pools["consts"]
            wtile = consts.tile([P, 2, C], FP8, name="wtile", tag="wtile")
            nc.vector.memset(wtile, 0.03125)
            wps = None
            for i in range(N_WARM):
                wps = self.psum1(f"warm_ps_{i}")
                nc.tensor.matmul(
                    wps, lhsT=wtile[:, :, 0:P], rhs=wtile,
                    start=True, stop=True, perf_mode=DR,
                )
            wsb = small.tile([P, C], F32, name="warm_sb", tag="warm_sb", bufs=1)
            nc.scalar.copy(wsb, wps)
            nc.sync.dma_start(out=self.scratch_d, in_=wsb)

        mult = mybir.AluOpType.mult
        add = mybir.AluOpType.add
        mv_all = small.tile([P, GROUPS, 2], F32, name=f"gnmv_all_{r}", tag="gnmv_all")
        for g in range(GROUPS):
            stats = small.tile([P, 4, 6], F32, name=f"gnstats_{g}_{r}", tag=f"gnstats_{g}")
            for s in range(4):
                nc.vector.bn_stats(out=stats[:, s, :], in_=x_sb[g][:, s * 512 : (s + 1) * 512])
            mv = mv_all[:, g, :]
            nc.vector.bn_aggr(out=mv, in_=stats)
            # mv = [mean_p, var_p] per partition; mv[:,1] <- var_p + mean_p^2
            nc.vector.scalar_tensor_tensor(
                out=mv[:, 1:2], in0=mv[:, 0:1], scalar=mv[:, 0:1], in1=mv[:, 1:2],
                op0=mult, op1=add,
            )
        st["mv_all"] = mv_all
        return st

    # ---- head part 2: cross-partition reduce (one matmul pair), A/B chain,
    # fp8 casts ----
    def head_post(self, st):
        nc = self.nc
        small = self.pools["small"]
        persist = self.pools["persist"]
        ones_bc, ones_col, ones_row, eps_t = self._consts
        Sqrt = mybir.ActivationFunctionType.Sqrt
        mult = mybir.AluOpType.mult
        add = mybir.AluOpType.add
        cv = st["cv"]
        gnw_sb = cv[:, 2, :]
        gnb_sb = cv[:, 3, :]

        gsum_ps = self.psum1("gsum_ps")
        nc.tensor.matmul(gsum_ps[0:1, 0:8], lhsT=ones_col,
                         rhs=st["mv_all"].rearrange("p g two -> p (g two)"),
                         start=True, stop=True)
        gsum = small.tile([1, 8], F32, name="gsum", tag="gsum")
        nc.scalar.copy(gsum, gsum_ps[0:1, 0:8])
        gbc_ps = self.psum1("gbc_ps")
        nc.tensor.matmul(gbc_ps[:, 0:8], lhsT=ones_row, rhs=gsum, start=True, stop=True)
        mvb = small.tile([P, 8], F32, name="gnmvb", tag="gnmvb")
        nc.vector.tensor_scalar_mul(mvb, gbc_ps[:, 0:8], 1.0 / P)
        mean_a = mvb[:, 0:8:2]  # [128, 4]
        m2_a = mvb[:, 1:8:2]
        msq = small.tile([P, 4], F32, name="gnmsq", tag="gnmsq")
        nc.vector.tensor_mul(msq, mean_a, mean_a)
        varg = small.tile([P, 4], F32, name="gnvar", tag="gnvar")
        nc.vector.tensor_sub(varg, m2_a, msq)
        stdg = small.tile([P, 4], F32, name="gnstd", tag="gnstd")
        nc.scalar.activation(stdg, varg, Sqrt, bias=eps_t)
        rstd = small.tile([P, 4], F32, name="gnrstd", tag="gnrstd")
        nc.vector.reciprocal(rstd, stdg)
        a_t = small.tile([P, 4], F32, name="gnA", tag="gnA")
        nc.vector.tensor_mul(a_t, rstd, gnw_sb)
        ma_t = small.tile([P, 4], F32, name="gnmA", tag="gnmA")
        nc.vector.tensor_mul(ma_t, mean_a, a_t)
        b_t = small.tile([P, 4], F32, name="gnB", tag="gnB")
        nc.vector.tensor_sub(b_t, gnb_sb, ma_t)
        xn = [
            persist.tile([P, 2, L], FP8, tag=f"xn_{p}", name=f"xn2_{p}", bufs=2)
            for p in range(2)
        ]
        for g in range(GROUPS):
            # xn = fp8(x*A + B), written into pair tile [128, 2, L]
            nc.vector.tensor_scalar(
                out=xn[g // 2][:, g % 2, :], in0=st["x_sb"][g],
                scalar1=a_t[:, g : g + 1], scalar2=b_t[:, g : g + 1],
                op0=mult, op1=add,
            )
        st["xn"] = xn

    # ---- Q, K: w-stationary; ScalarE takes the plain k copies (idle pre-S),
    # DVE the q bias-add, keeping ScalarE exp-only during the S phase ----
    def attn_qk(self, st):
        nc = self.nc
        persist = self.pools["persist"]
        add = mybir.AluOpType.add
        wall, cv, xn = st["wall"], st["cv"], st["xn"]
        qb_sb = cv[:, 0, :]

        def w2(pname, pr):
            return wall[:, PROJ_IDX[pname], pr]

        st["w2"] = w2
        q2 = [persist.tile([P, 2, L], FP8, name=f"q2_{p}", tag=f"q2_{p}", bufs=1) for p in range(2)]
        k2 = [persist.tile([P, 2, L], FP8, name=f"k2_{p}", tag=f"k2_{p}", bufs=1) for p in range(2)]
        for ot in range(NCT):
            for pname, dest in (("q", q2), ("k", k2)):
                t = dest[ot // 2]
                for half in range(2):
                    ps = self.psum2(f"qk_ps_{pname}_{ot}_{half}")
                    for pr in range(2):
                        for s in range(2):
                            nc.tensor.matmul(
                                ps[:, s * 512 : (s + 1) * 512],
                                lhsT=w2(pname, pr)[:, :, ot * P : (ot + 1) * P],
                                rhs=xn[pr][:, :, (2 * half + s) * 512 : (2 * half + s + 1) * 512],
                                start=(pr == 0),
                                stop=(pr == 1),
                                perf_mode=DR,
                            )
                    dst = t[:, ot % 2, half * 1024 : (half + 1) * 1024]
                    if pname == "q":
                        nc.vector.tensor_scalar(
                            out=dst, in0=ps, scalar1=qb_sb[:, ot : ot + 1],
                            scalar2=None, op0=add,
                        )
                    else:
                        nc.scalar.copy(dst, ps)
        st["q2"], st["k2"] = q2, k2

    # ---- S^T = K^T Q per j-tile + exp; V^T matmuls interleaved as PE filler ----
    def attn_s(self, st):
        nc = self.nc
        persist = self.pools["persist"]
        Exp = mybir.ActivationFunctionType.Exp
        q2, k2, xn, w2 = st["q2"], st["k2"], st["xn"], st["w2"]

        vt2 = [
            persist.tile([P, 2, C], FP8, name=f"vt2_{p}", tag=f"vt2_{p}", bufs=1)
            for p in range(NLT // 2)
        ]
        e2 = [
            persist.tile([P, 2, L], FP8, tag=f"e2_{jp}", name=f"e2_{jp}", bufs=1)
            for jp in range(NLT // 2)
        ]
        for jt in range(NLT):
            jp, sub = divmod(jt, 2)
            for half in range(2):
                ps = self.psum2(f"s_ps_{jt}_{half}")
                for pr in range(2):
                    for s in range(2):
                        nc.tensor.matmul(
                            ps[:, s * 512 : (s + 1) * 512],
                            lhsT=k2[pr][:, :, jt * P : (jt + 1) * P],
                            rhs=q2[pr][:, :, (2 * half + s) * 512 : (2 * half + s + 1) * 512],
                            start=(pr == 0),
                            stop=(pr == 1),
                            perf_mode=DR,
                        )
                nc.scalar.activation(
                    e2[jp][:, sub, half * 1024 : (half + 1) * 1024], ps, Exp, scale=SCALE
                )
            # V^T for l-tile jt: 2 matmuls into a [128,512] psum, then copyback
            psv = self.psum1(f"vt_ps_{jt}")
            for pr in range(2):
                nc.tensor.matmul(
                    psv,
                    lhsT=xn[pr][:, :, jt * P : (jt + 1) * P],
                    rhs=w2("v", pr),
                    start=(pr == 0),
                    stop=(pr == 1),
                    perf_mode=DR,
                )
            nc.vector.tensor_copy(vt2[jp][:, sub, :], psv)
        st["vt2"], st["e2"] = vt2, e2

    # ---- d, O, proj + bias + residual + store ----
    def attn_rest(self, st):
        nc = self.nc
        persist = self.pools["persist"]
        fin_pool = self.pools["fin"]
        ones_bc = self._consts[0]
        mult = mybir.AluOpType.mult
        add = mybir.AluOpType.add
        e2, vt2, w2, cv = st["e2"], st["vt2"], st["w2"], st["cv"]
        pb_sb = cv[:, 1, :]

        # d[i] = sum_j E[j,i]: DoubleRow ones-matmuls, ones tile stationary
        # across all 32; every psum row holds d broadcast; reciprocal on DVE.
        dinvb = persist.tile([P, L], F32, tag="dinvb", name="dinvb", bufs=1)
        for ib in range(4):
            dps = self.psum1(f"d_ps_{ib}")
            for jp in range(NLT // 2):
                nc.tensor.matmul(
                    dps,
                    lhsT=ones_bc,
                    rhs=e2[jp][:, :, ib * 512 : (ib + 1) * 512],
                    start=(jp == 0),
                    stop=(jp == NLT // 2 - 1),
                    perf_mode=DR,
                )
            nc.vector.reciprocal(dinvb[:, ib * 512 : (ib + 1) * 512], dps)

        # O_unnorm[c, i] = sum_j VT[j, c] E[j, i], i-half-major per ct;
        # copyback folds 1/d and casts to fp8
        o2 = [persist.tile([P, 2, L], FP8, tag=f"o2_{p}", name=f"o2_{p}", bufs=1) for p in range(2)]
        for ct in range(NCT):
            t = o2[ct // 2]
            for half in range(2):
                po = self.psum2(f"o_ps_{ct}_{half}")
                for jp in range(NLT // 2):
                    for s in range(2):
                        nc.tensor.matmul(
                            po[:, s * 512 : (s + 1) * 512],
                            lhsT=vt2[jp][:, :, ct * P : (ct + 1) * P],
                            rhs=e2[jp][:, :, (2 * half + s) * 512 : (2 * half + s + 1) * 512],
                            start=(jp == 0),
                            stop=(jp == NLT // 2 - 1),
                            perf_mode=DR,
                        )
                nc.vector.tensor_mul(
                    t[:, ct % 2, half * 1024 : (half + 1) * 1024],
                    po,
                    dinvb[:, half * 1024 : (half + 1) * 1024],
                )

        # proj + bias + residual + store
        for ot in range(NCT):
            for half in range(2):
                pp = self.psum2(f"p_ps_{ot}_{half}")
                for pr in range(2):
                    for s in range(2):
                        nc.tensor.matmul(
                            pp[:, s * 512 : (s + 1) * 512],
                            lhsT=w2("p", pr)[:, :, ot * P : (ot + 1) * P],
                            rhs=o2[pr][:, :, (2 * half + s) * 512 : (2 * half + s + 1) * 512],
                            start=(pr == 0),
                            stop=(pr == 1),
                            perf_mode=DR,
                        )
                fo = fin_pool.tile([P, 1024], F32, tag="fo", name=f"fo_{ot}_{half}")
                nc.vector.scalar_tensor_tensor(
                    out=fo, in0=pp, scalar=pb_sb[:, ot : ot + 1],
                    in1=st["x_sb"][ot][:, half * 1024 : (half + 1) * 1024],
                    op0=add, op1=add,
                )
                nc.sync.dma_start(
                    out=self.out_d[ot * P : (ot + 1) * P, half * 1024 : (half + 1) * 1024],
                    in_=fo,
                )


_NC_CACHE = None


def _get_program():
    global _NC_CACHE
    if _NC_CACHE is None:
        _NC_CACHE = build_program()
    return _NC_CACHE


def make_in_maps(x, gn_w, gn_b, qw, qb, kw, kb, vw, vb, pw, pb):
    import ml_dtypes

    f = np.float32
    f8 = ml_dtypes.float8_e4m3

    def pair_w(w):
        # w [Cout, Cin] -> wT [Cin, Cout] -> [2(pair), 128(k), 2(sub), Cout] fp8
        wT = np.asarray(w, f).T.reshape(2, 2, P, C).transpose(0, 2, 1, 3)
        return wT.astype(f8)

    # all weights in one [128, 4(proj), 2(pair), 2(sub), C] tensor
    wall = np.stack(
        [pair_w(qw), pair_w(kw), pair_w(vw), pair_w(pw)], axis=0
    ).transpose(2, 0, 1, 3, 4)  # [P, proj, pr, sub, C]

    pb_eff = np.asarray(pb, f) + np.asarray(pw, f) @ np.asarray(vb, f)
    # channel vectors packed [128, 4(vec), 4(group-of-128)]
    cvecs = np.stack(
        [np.asarray(qb, f), pb_eff, np.asarray(gn_w, f), np.asarray(gn_b, f)], axis=0
    ).reshape(4, NCT, P).transpose(2, 0, 1)

    shared = {
        "wall": np.ascontiguousarray(wall),
        "cvecs": np.ascontiguousarray(cvecs),
    }
    x = np.asarray(x, f)
    return [{"x": np.ascontiguousarray(x[b]), **shared} for b in range(B)]


def kernel(x, gn_w, gn_b, qw, qb, kw, kb, vw, vb, pw, pb):
    nc = _get_program()
    in_maps = make_in_maps(x, gn_w, gn_b, qw, qb, kw, kb, vw, vb, pw, pb)
    res = run_bass_kernel_spmd(nc, in_maps, core_ids=list(range(B)))
    return np.stack([res.results[b]["out"] for b in range(B)]).astype(np.float32)
